# revision 2
# baseline (speedup 1.0000x reference)
"""Trainium2 Bass kernel for the 3-layer LCN/GNN network, PE-dense strategy.

Each layer out[b,d] = relu(sum_k x[b, knn[d,k]] * w[d,k] + b[d]) is
expressed as a dense matmul x @ S_l where S_l[f, d] holds w[d, k] at the
16 scattered rows f = knn[d, k] and exact zeros elsewhere (bf16; zeros
stay exact so only true terms contribute; accumulation is fp32 PSUM).

Sharding: model-parallel over output nodes (1/8 per core), full batch
everywhere. Per core and layer: stationary lhsT = actT tiles [128f, 128b]
(SBUF-resident), moving rhs = the S_l column shard streamed from HBM in
256KB k-slabs; psum [128b, <=512] accumulates over all k-tiles; a 1-row
ones matmul adds the bias row; ACT applies ReLU; the xbar DMA transpose
flips [128b, nodes] into [node, 256b] tiles which go to a DRAM shard and
are AllGather'd into the next layer's full actT table.

FC head: per-core partial y^T[3, 256] over its 256 L2 nodes; the 8
partials are summed on the host (24 values/sample) and fc_b added there.
"""

import os
import sys
import types

import numpy as np

try:  # pragma: no cover
    import antenv.axon_hooks  # noqa: F401
except Exception:
    _m = types.ModuleType("antenv.axon_hooks")
    _m.get_axon_ntff_profile_hook = lambda: None
    sys.modules["antenv.axon_hooks"] = _m

B, IN_DIM, K = 256, 16384, 16
DIMS = [8192, 4096, 2048]
PREV = [IN_DIM] + DIMS[:-1]
OUT_DIM = 3
N_CORES = 8
P = 128

_cache = {}


def _build(reps: int = 1):
    import concourse.tile as tile
    from concourse import bacc, mybir

    nc = bacc.Bacc("TRN2", target_bir_lowering=False, debug=False,
                   num_devices=N_CORES)
    f32 = mybir.dt.float32
    bf16 = mybir.dt.bfloat16

    shard = [d // N_CORES for d in DIMS]  # 1024, 512, 256
    groups = [list(range(N_CORES))]

    t0 = nc.dram_tensor("t0", [IN_DIM, B], bf16, kind="ExternalInput")
    t1 = nc.dram_tensor("t1", [DIMS[0], B], bf16, addr_space="Shared")
    t2 = nc.dram_tensor("t2", [DIMS[1], B], bf16, addr_space="Shared")
    sh1 = nc.dram_tensor("sh1", [shard[0], B], bf16)
    sh2 = nc.dram_tensor("sh2", [shard[1], B], bf16)
    shards = [sh1, sh2]
    tnexts = [t1, t2]

    s_d = [nc.dram_tensor(f"s{l}", [PREV[l], shard[l]], bf16,
                          kind="ExternalInput") for l in range(3)]
    bias_d = [nc.dram_tensor(f"bias{l}", [1, shard[l]], bf16,
                             kind="ExternalInput") for l in range(3)]
    ones_d = nc.dram_tensor("ones", [1, P], bf16, kind="ExternalInput")
    fcw_d = nc.dram_tensor("fcw", [P, 2 * OUT_DIM], bf16, kind="ExternalInput")
    out_d = nc.dram_tensor("out", [OUT_DIM, B], f32, kind="ExternalOutput")

    with tile.TileContext(nc) as tc:
        with (
            tc.tile_pool(name="const", bufs=1) as cpool,
            tc.tile_pool(name="wstr", bufs=3) as wpool,
            tc.tile_pool(name="acts", bufs=2) as apool,
            tc.tile_pool(name="psum", bufs=1, space="PSUM") as ppool,
        ):
            a0 = cpool.tile([P, IN_DIM // P, B], bf16, tag="a0")
            a1 = cpool.tile([P, DIMS[0] // P, B], bf16, tag="a1")
            a2 = cpool.tile([P, DIMS[1] // P, B], bf16, tag="a2")
            acts = [a0, a1, a2]
            nc.sync.dma_start(
                out=a0[:], in_=t0.ap().rearrange("(c p) b -> p c b", p=P))
            ones = cpool.tile([1, P], bf16, tag="ones")
            nc.sync.dma_start(out=ones[:], in_=ones_d.ap())
            bias_sb = []
            for l in range(3):
                bt = cpool.tile([1, shard[l]], bf16, tag=f"bias{l}")
                nc.sync.dma_start(out=bt[:], in_=bias_d[l].ap())
                bias_sb.append(bt)
            fcw_sb = cpool.tile([P, 2 * OUT_DIM], bf16, tag="fcw")
            nc.sync.dma_start(out=fcw_sb[:], in_=fcw_d.ap())
            act2T = cpool.tile([P, 2, B], bf16, tag="act2T")

            def emit_net():
                for l in range(3):
                    kt = PREV[l] // P
                    ns = shard[l]
                    at = acts[l]
                    nch = (ns + 511) // 512
                    # psum tiles [128b, <=512] per (batch-half, col-chunk)
                    pss = {}
                    for m in range(2):
                        for n in range(nch):
                            nw = min(512, ns - n * 512)
                            pss[m, n] = (
                                ppool.tile([P, nw], f32, tag=f"ps{m}{n}",
                                           name=f"ps{m}{n}"), nw)
                    for ki in range(kt):
                        st = wpool.tile([P, ns], bf16, tag="S")
                        nc.sync.dma_start(
                            out=st[:], in_=s_d[l][ki * P:(ki + 1) * P, :])
                        for m in range(2):
                            for n in range(nch):
                                ps, nw = pss[m, n]
                                nc.tensor.matmul(
                                    out=ps[:],
                                    lhsT=at[:, ki, m * 128:(m + 1) * 128],
                                    rhs=st[:, n * 512:n * 512 + nw],
                                    start=(ki == 0),
                                    stop=False,
                                )
                    arows = []
                    for m in range(2):
                        for n in range(nch):
                            ps, nw = pss[m, n]
                            nc.tensor.matmul(
                                out=ps[:],
                                lhsT=ones[:],
                                rhs=bias_sb[l][:, n * 512:n * 512 + nw],
                                start=False,
                                stop=True,
                            )
                        arow = apool.tile([P, ns], bf16, tag=f"ar{m}")
                        for n in range(nch):
                            ps, nw = pss[m, n]
                            nc.scalar.activation(
                                out=arow[:, n * 512:n * 512 + nw],
                                in_=ps[:],
                                func=mybir.ActivationFunctionType.Relu,
                                scale=1.0,
                            )
                        arows.append(arow)
                    for n in range(ns // P):
                        if l < 2:
                            dst = apool.tile([P, B], bf16, tag="T")
                            d0, d1 = dst[:, 0:128], dst[:, 128:256]
                        else:
                            d0, d1 = act2T[:, n, 0:128], act2T[:, n, 128:256]
                        nc.sync.dma_start(
                            out=d0, in_=arows[0][:, n * P:(n + 1) * P],
                            transpose=True)
                        nc.sync.dma_start(
                            out=d1, in_=arows[1][:, n * P:(n + 1) * P],
                            transpose=True)
                        if l < 2:
                            nc.sync.dma_start(
                                out=shards[l][n * P:(n + 1) * P, :],
                                in_=dst[:])
                    if l < 2:
                        nc.gpsimd.collective_compute(
                            "AllGather",
                            mybir.AluOpType.bypass,
                            groups,
                            ins=[shards[l].ap()],
                            outs=[tnexts[l].ap()],
                        )
                        nc.sync.dma_start(
                            out=acts[l + 1][:],
                            in_=tnexts[l].ap().rearrange(
                                "(c p) b -> p c b", p=P))

                ps = ppool.tile([OUT_DIM, B], f32, tag="fc")
                for t in range(2):
                    nc.tensor.matmul(
                        out=ps[:],
                        lhsT=fcw_sb[:, t * OUT_DIM:(t + 1) * OUT_DIM],
                        rhs=act2T[:, t, :],
                        start=(t == 0),
                        stop=(t == 1),
                    )
                fin = apool.tile([OUT_DIM, B], f32, tag="fin")
                nc.vector.tensor_copy(out=fin[:], in_=ps[:])
                nc.sync.dma_start(out=out_d.ap(), in_=fin[:])

            for r in range(reps):
                if r:
                    tc.strict_bb_all_engine_barrier()
                emit_net()

    nc.compile()
    return nc


def _prep_inputs(inputs):
    import ml_dtypes

    shard = [d // N_CORES for d in DIMS]
    x = np.asarray(inputs["x"], dtype=np.float32)
    t0 = np.ascontiguousarray(x.T).astype(ml_dtypes.bfloat16)
    fcw = np.asarray(inputs["fc_w"], dtype=np.float32)
    ones = np.ones((1, P), dtype=ml_dtypes.bfloat16)

    in_maps = []
    for m in range(N_CORES):
        im = {"t0": t0, "ones": ones}
        for l, d in enumerate(DIMS):
            knn = np.asarray(inputs[f"knn{l}"], dtype=np.int64)
            w = np.asarray(inputs[f"w{l}"], dtype=np.float32)
            b = np.asarray(inputs[f"b{l}"], dtype=np.float32).reshape(d)
            lo = m * shard[l]
            nodes = np.arange(lo, lo + shard[l])
            # Accumulate (duplicate knn entries per node must sum).
            s32 = np.zeros((PREV[l], shard[l]), dtype=np.float32)
            rows = knn[nodes].ravel()
            cols = np.repeat(np.arange(shard[l]), K)
            np.add.at(s32, (rows, cols), w[nodes].ravel())
            im[f"s{l}"] = s32.astype(ml_dtypes.bfloat16)
            im[f"bias{l}"] = b[lo:lo + shard[l]].reshape(1, -1).astype(
                ml_dtypes.bfloat16)
        cols = fcw[:, m * 256:(m + 1) * 256].T
        im["fcw"] = np.ascontiguousarray(
            cols.reshape(2, P, OUT_DIM).transpose(1, 0, 2).reshape(P, 2 * OUT_DIM)
        ).astype(ml_dtypes.bfloat16)
        in_maps.append(im)
    return in_maps


def kernel(**inputs) -> np.ndarray:
    from concourse.bass_utils import run_bass_kernel_spmd

    reps = int(os.environ.get("KERNEL_REPS", "1"))
    key = ("nc", reps)
    if key not in _cache:
        _cache[key] = _build(reps)
    nc = _cache[key]

    in_maps = _prep_inputs(inputs)
    res = run_bass_kernel_spmd(nc, in_maps, list(range(N_CORES)))
    if res.exec_time_ns is not None:
        print(f"HW exec time: {res.exec_time_ns} ns")
    acc = np.zeros((OUT_DIM, B), dtype=np.float32)
    for r in res.results:
        acc += r["out"]
    fc_b = np.asarray(inputs["fc_b"], dtype=np.float32)
    return (acc.T + fc_b[None, :]).astype(np.float32)


if __name__ == "__main__":
    sys.path.insert(0, "/root/problem")
    inputs = dict(np.load("/root/problem/inputs.npz"))
    expected = np.load("/root/problem/expected.npy")
    actual = kernel(**inputs)
    err = np.abs(actual - expected)
    scale = np.abs(expected).max()
    print(f"absmax err: {err.max():.6g}  scale: {scale:.6g}")
    print(f"Relative error: {err.max() / scale:.6g}")


# revision 3
# speedup vs baseline: 1.0367x; 1.0367x over previous
"""Trainium2 Bass kernel for the 3-layer LCN/GNN network, PE-dense strategy
with split-half AllGather/compute overlap.

As kernel_e (dense scattered-weight matmuls x @ S_l, model-parallel over
nodes, bf16, fp32 PSUM), but each of layers 0/1 computes its output
columns in two half-passes over the streamed S shard: half A's
bias/ReLU/transpose/AllGather runs on ACT/DMA/CCOM while the PE streams
half B, hiding most of the first collective; the next layer's contraction
is ordered so the k-tiles produced by AllGather(A) are consumed first,
hiding part of AllGather(B) as well. Next-layer tables use the
[cores x halfA | cores x halfB] row order; S_{l+1} rows are permuted on
the host to match.
"""

import os
import sys
import types

import numpy as np

try:  # pragma: no cover
    import antenv.axon_hooks  # noqa: F401
except Exception:
    _m = types.ModuleType("antenv.axon_hooks")
    _m.get_axon_ntff_profile_hook = lambda: None
    sys.modules["antenv.axon_hooks"] = _m

B, IN_DIM, K = 256, 16384, 16
DIMS = [8192, 4096, 2048]
PREV = [IN_DIM] + DIMS[:-1]
OUT_DIM = 3
N_CORES = 8
P = 128

_cache = {}


def _perm(dim):
    """Row order of the gathered table for a layer with `dim` total nodes:
    all cores' first half-shards, then all cores' second half-shards."""
    sh = dim // N_CORES
    h = sh // 2
    order = [c * sh + j for c in range(N_CORES) for j in range(h)]
    order += [c * sh + h + j for c in range(N_CORES) for j in range(h)]
    return np.asarray(order, dtype=np.int64)


def _build(reps: int = 1):
    import concourse.tile as tile
    from concourse import bacc, mybir

    nc = bacc.Bacc("TRN2", target_bir_lowering=False, debug=False,
                   num_devices=N_CORES)
    f32 = mybir.dt.float32
    bf16 = mybir.dt.bfloat16

    shard = [d // N_CORES for d in DIMS]  # 1024, 512, 256
    groups = [list(range(N_CORES))]

    t0 = nc.dram_tensor("t0", [IN_DIM, B], bf16, kind="ExternalInput")
    # Gathered half-tables (AllGather outputs) per layer boundary.
    tn = [[nc.dram_tensor(f"t{l + 1}{'ab'[h]}", [DIMS[l] // 2, B], bf16,
                          addr_space="Shared") for h in range(2)]
          for l in range(2)]
    shd = [[nc.dram_tensor(f"sh{l + 1}{'ab'[h]}", [shard[l] // 2, B], bf16)
            for h in range(2)] for l in range(2)]

    # S shards: layers 0/1 stored as [2*PREV, shard/2] (column halves
    # stacked), layer 2 as [PREV, shard].
    s_d = [
        nc.dram_tensor("s0", [2 * PREV[0], shard[0] // 2], bf16,
                       kind="ExternalInput"),
        nc.dram_tensor("s1", [2 * PREV[1], shard[1] // 2], bf16,
                       kind="ExternalInput"),
        nc.dram_tensor("s2", [PREV[2], shard[2]], bf16,
                       kind="ExternalInput"),
    ]
    bias_d = [nc.dram_tensor(f"bias{l}", [1, shard[l]], bf16,
                             kind="ExternalInput") for l in range(3)]
    ones_d = nc.dram_tensor("ones", [1, P], bf16, kind="ExternalInput")
    fcw_d = nc.dram_tensor("fcw", [P, 2 * OUT_DIM], bf16, kind="ExternalInput")
    out_d = nc.dram_tensor("out", [OUT_DIM, B], f32, kind="ExternalOutput")

    with tile.TileContext(nc) as tc:
        with (
            tc.tile_pool(name="const", bufs=1) as cpool,
            tc.tile_pool(name="wstr", bufs=4) as wpool,
            tc.tile_pool(name="acts", bufs=2) as apool,
            tc.tile_pool(name="psum", bufs=1, space="PSUM") as ppool,
        ):
            a0 = cpool.tile([P, IN_DIM // P, B], bf16, tag="a0")
            a1 = cpool.tile([P, DIMS[0] // P, B], bf16, tag="a1")
            a2 = cpool.tile([P, DIMS[1] // P, B], bf16, tag="a2")
            acts = [a0, a1, a2]
            nc.sync.dma_start(
                out=a0[:], in_=t0.ap().rearrange("(c p) b -> p c b", p=P))
            ones = cpool.tile([1, P], bf16, tag="ones")
            nc.sync.dma_start(out=ones[:], in_=ones_d.ap())
            bias_sb = []
            for l in range(3):
                bt = cpool.tile([1, shard[l]], bf16, tag=f"bias{l}")
                nc.sync.dma_start(out=bt[:], in_=bias_d[l].ap())
                bias_sb.append(bt)
            fcw_sb = cpool.tile([P, 2 * OUT_DIM], bf16, tag="fcw")
            nc.sync.dma_start(out=fcw_sb[:], in_=fcw_d.ap())
            act2T = cpool.tile([P, 2, B], bf16, tag="act2T")

            def emit_net():
                for l in range(3):
                    kt = PREV[l] // P
                    ns = shard[l]
                    at = acts[l]
                    H = 2 if l < 2 else 1
                    nsh = ns // H
                    for h in range(H):
                        # accumulate this half's columns over all k-tiles;
                        # stream S in 2-ktile (256KB) slabs
                        pss = {}
                        for m in range(2):
                            pss[m] = ppool.tile([P, nsh], f32, tag=f"ps{m}",
                                                name=f"ps{m}")
                        for kj in range(kt // 2):
                            st = wpool.tile([P, 2, nsh], bf16, tag="S")
                            r0 = h * PREV[l] + kj * 256
                            nc.sync.dma_start(
                                out=st[:],
                                in_=s_d[l][r0:r0 + 256, :].rearrange(
                                    "(two p) n -> p two n", p=P))
                            for jj in range(2):
                                ki = kj * 2 + jj
                                for m in range(2):
                                    nc.tensor.matmul(
                                        out=pss[m][:],
                                        lhsT=at[:, ki, m * 128:(m + 1) * 128],
                                        rhs=st[:, jj, :],
                                        start=(ki == 0),
                                        stop=False,
                                    )
                        arows = []
                        for m in range(2):
                            nc.tensor.matmul(
                                out=pss[m][:],
                                lhsT=ones[:],
                                rhs=bias_sb[l][:, h * nsh:(h + 1) * nsh],
                                start=False,
                                stop=True,
                            )
                            arow = apool.tile([P, nsh], bf16, tag=f"ar{m}")
                            nc.scalar.activation(
                                out=arow[:],
                                in_=pss[m][:],
                                func=mybir.ActivationFunctionType.Relu,
                                scale=1.0,
                            )
                            arows.append(arow)
                        for n in range(nsh // P):
                            if l < 2:
                                dst = apool.tile([P, B], bf16, tag="T")
                                d0, d1 = dst[:, 0:128], dst[:, 128:256]
                            else:
                                g = h * (nsh // P) + n
                                d0 = act2T[:, g, 0:128]
                                d1 = act2T[:, g, 128:256]
                            nc.sync.dma_start(
                                out=d0, in_=arows[0][:, n * P:(n + 1) * P],
                                transpose=True)
                            nc.sync.dma_start(
                                out=d1, in_=arows[1][:, n * P:(n + 1) * P],
                                transpose=True)
                            if l < 2:
                                nc.sync.dma_start(
                                    out=shd[l][h][n * P:(n + 1) * P, :],
                                    in_=dst[:])
                        if l < 2:
                            nc.gpsimd.collective_compute(
                                "AllGather",
                                mybir.AluOpType.bypass,
                                groups,
                                ins=[shd[l][h].ap()],
                                outs=[tn[l][h].ap()],
                            )
                            ktn_h = DIMS[l] // 2 // P
                            nc.sync.dma_start(
                                out=acts[l + 1][:, h * ktn_h:(h + 1) * ktn_h, :],
                                in_=tn[l][h].ap().rearrange(
                                    "(c p) b -> p c b", p=P))

                ps = ppool.tile([OUT_DIM, B], f32, tag="fc")
                for t in range(2):
                    nc.tensor.matmul(
                        out=ps[:],
                        lhsT=fcw_sb[:, t * OUT_DIM:(t + 1) * OUT_DIM],
                        rhs=act2T[:, t, :],
                        start=(t == 0),
                        stop=(t == 1),
                    )
                fin = apool.tile([OUT_DIM, B], f32, tag="fin")
                nc.vector.tensor_copy(out=fin[:], in_=ps[:])
                nc.sync.dma_start(out=out_d.ap(), in_=fin[:])

            for r in range(reps):
                if r:
                    tc.strict_bb_all_engine_barrier()
                emit_net()

    nc.compile()
    return nc


def _prep_inputs(inputs):
    import ml_dtypes

    shard = [d // N_CORES for d in DIMS]
    x = np.asarray(inputs["x"], dtype=np.float32)
    t0 = np.ascontiguousarray(x.T).astype(ml_dtypes.bfloat16)
    fcw = np.asarray(inputs["fc_w"], dtype=np.float32)
    ones = np.ones((1, P), dtype=ml_dtypes.bfloat16)
    perms = [None, _perm(DIMS[0]), _perm(DIMS[1])]  # row order of t_l inputs

    in_maps = []
    for m in range(N_CORES):
        im = {"t0": t0, "ones": ones}
        for l, d in enumerate(DIMS):
            knn = np.asarray(inputs[f"knn{l}"], dtype=np.int64)
            w = np.asarray(inputs[f"w{l}"], dtype=np.float32)
            b = np.asarray(inputs[f"b{l}"], dtype=np.float32).reshape(d)
            lo = m * shard[l]
            nodes = np.arange(lo, lo + shard[l])
            s32 = np.zeros((PREV[l], shard[l]), dtype=np.float32)
            rows = knn[nodes].ravel()
            cols = np.repeat(np.arange(shard[l]), K)
            np.add.at(s32, (rows, cols), w[nodes].ravel())
            if perms[l] is not None:
                s32 = s32[perms[l]]  # match the gathered-table row order
            if l < 2:
                half = shard[l] // 2
                s32 = np.concatenate([s32[:, :half], s32[:, half:]], axis=0)
            im[f"s{l}"] = np.ascontiguousarray(s32).astype(ml_dtypes.bfloat16)
            im[f"bias{l}"] = b[lo:lo + shard[l]].reshape(1, -1).astype(
                ml_dtypes.bfloat16)
        cols = fcw[:, m * 256:(m + 1) * 256].T
        im["fcw"] = np.ascontiguousarray(
            cols.reshape(2, P, OUT_DIM).transpose(1, 0, 2).reshape(P, 2 * OUT_DIM)
        ).astype(ml_dtypes.bfloat16)
        in_maps.append(im)
    return in_maps


def kernel(**inputs) -> np.ndarray:
    from concourse.bass_utils import run_bass_kernel_spmd

    reps = int(os.environ.get("KERNEL_REPS", "1"))
    key = ("nc", reps)
    if key not in _cache:
        _cache[key] = _build(reps)
    nc = _cache[key]

    in_maps = _prep_inputs(inputs)
    res = run_bass_kernel_spmd(nc, in_maps, list(range(N_CORES)))
    if res.exec_time_ns is not None:
        print(f"HW exec time: {res.exec_time_ns} ns")
    acc = np.zeros((OUT_DIM, B), dtype=np.float32)
    for r in res.results:
        acc += r["out"]
    fc_b = np.asarray(inputs["fc_b"], dtype=np.float32)
    return (acc.T + fc_b[None, :]).astype(np.float32)


if __name__ == "__main__":
    sys.path.insert(0, "/root/problem")
    inputs = dict(np.load("/root/problem/inputs.npz"))
    expected = np.load("/root/problem/expected.npy")
    actual = kernel(**inputs)
    err = np.abs(actual - expected)
    scale = np.abs(expected).max()
    print(f"absmax err: {err.max():.6g}  scale: {scale:.6g}")
    print(f"Relative error: {err.max() / scale:.6g}")


# revision 4
# speedup vs baseline: 1.6575x; 1.5988x over previous
"""Trainium2 Bass kernel for the 3-layer LCN/GNN network, PE-dense strategy
with split-half AllGather/compute overlap.

As kernel_e (dense scattered-weight matmuls x @ S_l, model-parallel over
nodes, bf16, fp32 PSUM), but each of layers 0/1 computes its output
columns in two half-passes over the streamed S shard: half A's
bias/ReLU/transpose/AllGather runs on ACT/DMA/CCOM while the PE streams
half B, hiding most of the first collective; the next layer's contraction
is ordered so the k-tiles produced by AllGather(A) are consumed first,
hiding part of AllGather(B) as well. Next-layer tables use the
[cores x halfA | cores x halfB] row order; S_{l+1} rows are permuted on
the host to match.

Layer 0 additionally drops contraction rows that are all-zero in the
half's S_0 columns (mean hits/row is 0.5, so ~61% of rows vanish): the
host compacts each (core, half)'s used feature rows into a fixed
KT0H*128 window of a per-core t0 input, shrinking both the streamed S_0
bytes and the PE k-tile count by ~2.4x for layer 0.
"""

import os
import sys
import types

import numpy as np

try:  # pragma: no cover
    import antenv.axon_hooks  # noqa: F401
except Exception:
    _m = types.ModuleType("antenv.axon_hooks")
    _m.get_axon_ntff_profile_hook = lambda: None
    sys.modules["antenv.axon_hooks"] = _m

B, IN_DIM, K = 256, 16384, 16
DIMS = [8192, 4096, 2048]
PREV = [IN_DIM] + DIMS[:-1]
OUT_DIM = 3
N_CORES = 8
P = 128
KT0H = 54  # compacted L0 k-tiles per half (mean 50.4, +7 sigma)

_cache = {}


def _perm(dim):
    """Row order of the gathered table for a layer with `dim` total nodes:
    all cores' first half-shards, then all cores' second half-shards."""
    sh = dim // N_CORES
    h = sh // 2
    order = [c * sh + j for c in range(N_CORES) for j in range(h)]
    order += [c * sh + h + j for c in range(N_CORES) for j in range(h)]
    return np.asarray(order, dtype=np.int64)


def _build(reps: int = 1):
    import concourse.tile as tile
    from concourse import bacc, mybir

    nc = bacc.Bacc("TRN2", target_bir_lowering=False, debug=False,
                   num_devices=N_CORES)
    f32 = mybir.dt.float32
    bf16 = mybir.dt.bfloat16

    shard = [d // N_CORES for d in DIMS]  # 1024, 512, 256
    groups = [list(range(N_CORES))]

    t0 = nc.dram_tensor("t0", [2 * KT0H * P, B], bf16, kind="ExternalInput")
    # Gathered half-tables (AllGather outputs) per layer boundary.
    tn = [[nc.dram_tensor(f"t{l + 1}{'ab'[h]}", [DIMS[l] // 2, B], bf16,
                          addr_space="Shared") for h in range(2)]
          for l in range(2)]
    shd = [[nc.dram_tensor(f"sh{l + 1}{'ab'[h]}", [shard[l] // 2, B], bf16)
            for h in range(2)] for l in range(2)]

    # S shards: layers 0/1 stored as [2*PREV, shard/2] (column halves
    # stacked), layer 2 as [PREV, shard].
    s_d = [
        nc.dram_tensor("s0", [2 * KT0H * P, shard[0] // 2], bf16,
                       kind="ExternalInput"),
        nc.dram_tensor("s1", [2 * PREV[1], shard[1] // 2], bf16,
                       kind="ExternalInput"),
        nc.dram_tensor("s2", [PREV[2], shard[2]], bf16,
                       kind="ExternalInput"),
    ]
    bias_d = [nc.dram_tensor(f"bias{l}", [1, shard[l]], bf16,
                             kind="ExternalInput") for l in range(3)]
    ones_d = nc.dram_tensor("ones", [1, P], bf16, kind="ExternalInput")
    fcw_d = nc.dram_tensor("fcw", [P, 2 * OUT_DIM], bf16, kind="ExternalInput")
    out_d = nc.dram_tensor("out", [OUT_DIM, B], f32, kind="ExternalOutput")

    with tile.TileContext(nc) as tc:
        with (
            tc.tile_pool(name="const", bufs=1) as cpool,
            tc.tile_pool(name="wstr", bufs=4) as wpool,
            tc.tile_pool(name="acts", bufs=2) as apool,
            tc.tile_pool(name="psum", bufs=1, space="PSUM") as ppool,
        ):
            a0 = cpool.tile([P, 2 * KT0H, B], bf16, tag="a0")
            a1 = cpool.tile([P, DIMS[0] // P, B], bf16, tag="a1")
            a2 = cpool.tile([P, DIMS[1] // P, B], bf16, tag="a2")
            acts = [a0, a1, a2]
            nc.sync.dma_start(
                out=a0[:], in_=t0.ap().rearrange("(c p) b -> p c b", p=P))
            ones = cpool.tile([1, P], bf16, tag="ones")
            nc.sync.dma_start(out=ones[:], in_=ones_d.ap())
            bias_sb = []
            for l in range(3):
                bt = cpool.tile([1, shard[l]], bf16, tag=f"bias{l}")
                nc.sync.dma_start(out=bt[:], in_=bias_d[l].ap())
                bias_sb.append(bt)
            fcw_sb = cpool.tile([P, 2 * OUT_DIM], bf16, tag="fcw")
            nc.sync.dma_start(out=fcw_sb[:], in_=fcw_d.ap())
            act2T = cpool.tile([P, 2, B], bf16, tag="act2T")

            def emit_net():
                for l in range(3):
                    kt = KT0H if l == 0 else PREV[l] // P
                    ns = shard[l]
                    at = acts[l]
                    H = 2 if l < 2 else 1
                    nsh = ns // H
                    for h in range(H):
                        # accumulate this half's columns over all k-tiles;
                        # stream S in 2-ktile (256KB) slabs
                        pss = {}
                        for m in range(2):
                            pss[m] = ppool.tile([P, nsh], f32, tag=f"ps{m}",
                                                name=f"ps{m}")
                        for kj in range(kt // 2):
                            st = wpool.tile([P, 2, nsh], bf16, tag="S")
                            r0 = h * kt * P + kj * 256
                            nc.sync.dma_start(
                                out=st[:],
                                in_=s_d[l][r0:r0 + 256, :].rearrange(
                                    "(two p) n -> p two n", p=P))
                            for jj in range(2):
                                ki = kj * 2 + jj
                                kia = h * KT0H + ki if l == 0 else ki
                                for m in range(2):
                                    nc.tensor.matmul(
                                        out=pss[m][:],
                                        lhsT=at[:, kia, m * 128:(m + 1) * 128],
                                        rhs=st[:, jj, :],
                                        start=(ki == 0),
                                        stop=False,
                                    )
                        arows = []
                        for m in range(2):
                            nc.tensor.matmul(
                                out=pss[m][:],
                                lhsT=ones[:],
                                rhs=bias_sb[l][:, h * nsh:(h + 1) * nsh],
                                start=False,
                                stop=True,
                            )
                            arow = apool.tile([P, nsh], bf16, tag=f"ar{m}")
                            nc.scalar.activation(
                                out=arow[:],
                                in_=pss[m][:],
                                func=mybir.ActivationFunctionType.Relu,
                                scale=1.0,
                            )
                            arows.append(arow)
                        for n in range(nsh // P):
                            if l < 2:
                                dst = apool.tile([P, B], bf16, tag="T")
                                d0, d1 = dst[:, 0:128], dst[:, 128:256]
                            else:
                                g = h * (nsh // P) + n
                                d0 = act2T[:, g, 0:128]
                                d1 = act2T[:, g, 128:256]
                            nc.sync.dma_start(
                                out=d0, in_=arows[0][:, n * P:(n + 1) * P],
                                transpose=True)
                            nc.sync.dma_start(
                                out=d1, in_=arows[1][:, n * P:(n + 1) * P],
                                transpose=True)
                            if l < 2:
                                nc.sync.dma_start(
                                    out=shd[l][h][n * P:(n + 1) * P, :],
                                    in_=dst[:])
                        if l < 2:
                            nc.gpsimd.collective_compute(
                                "AllGather",
                                mybir.AluOpType.bypass,
                                groups,
                                ins=[shd[l][h].ap()],
                                outs=[tn[l][h].ap()],
                            )
                            ktn_h = DIMS[l] // 2 // P
                            nc.sync.dma_start(
                                out=acts[l + 1][:, h * ktn_h:(h + 1) * ktn_h, :],
                                in_=tn[l][h].ap().rearrange(
                                    "(c p) b -> p c b", p=P))

                ps = ppool.tile([OUT_DIM, B], f32, tag="fc")
                for t in range(2):
                    nc.tensor.matmul(
                        out=ps[:],
                        lhsT=fcw_sb[:, t * OUT_DIM:(t + 1) * OUT_DIM],
                        rhs=act2T[:, t, :],
                        start=(t == 0),
                        stop=(t == 1),
                    )
                fin = apool.tile([OUT_DIM, B], f32, tag="fin")
                nc.vector.tensor_copy(out=fin[:], in_=ps[:])
                nc.sync.dma_start(out=out_d.ap(), in_=fin[:])

            for r in range(reps):
                if r:
                    tc.strict_bb_all_engine_barrier()
                emit_net()

    nc.compile()
    return nc


def _prep_inputs(inputs):
    import ml_dtypes

    shard = [d // N_CORES for d in DIMS]
    x = np.asarray(inputs["x"], dtype=np.float32)
    t0 = np.ascontiguousarray(x.T).astype(ml_dtypes.bfloat16)
    fcw = np.asarray(inputs["fc_w"], dtype=np.float32)
    ones = np.ones((1, P), dtype=ml_dtypes.bfloat16)
    perms = [None, _perm(DIMS[0]), _perm(DIMS[1])]  # row order of t_l inputs

    in_maps = []
    for m in range(N_CORES):
        im = {"ones": ones}
        for l, d in enumerate(DIMS):
            knn = np.asarray(inputs[f"knn{l}"], dtype=np.int64)
            w = np.asarray(inputs[f"w{l}"], dtype=np.float32)
            b = np.asarray(inputs[f"b{l}"], dtype=np.float32).reshape(d)
            lo = m * shard[l]
            nodes = np.arange(lo, lo + shard[l])
            s32 = np.zeros((PREV[l], shard[l]), dtype=np.float32)
            rows = knn[nodes].ravel()
            cols = np.repeat(np.arange(shard[l]), K)
            np.add.at(s32, (rows, cols), w[nodes].ravel())
            if perms[l] is not None:
                s32 = s32[perms[l]]  # match the gathered-table row order
            if l == 0:
                # Per-half compaction: keep only rows used by this half's
                # columns, packed into a fixed KT0H*128 window, with the
                # matching x^T rows in a per-core t0.
                half = shard[0] // 2
                sblk, tblk = [], []
                for h in range(2):
                    sub = s32[:, h * half:(h + 1) * half]
                    used = np.flatnonzero(sub.any(axis=1))
                    assert len(used) <= KT0H * P, len(used)
                    sc = np.zeros((KT0H * P, half), dtype=np.float32)
                    sc[:len(used)] = sub[used]
                    tc = np.zeros((KT0H * P, B), dtype=ml_dtypes.bfloat16)
                    tc[:len(used)] = t0[used]
                    sblk.append(sc)
                    tblk.append(tc)
                im["t0"] = np.ascontiguousarray(np.concatenate(tblk, axis=0))
                s32 = np.concatenate(sblk, axis=0)
            elif l < 2:
                half = shard[l] // 2
                s32 = np.concatenate([s32[:, :half], s32[:, half:]], axis=0)
            im[f"s{l}"] = np.ascontiguousarray(s32).astype(ml_dtypes.bfloat16)
            im[f"bias{l}"] = b[lo:lo + shard[l]].reshape(1, -1).astype(
                ml_dtypes.bfloat16)
        cols = fcw[:, m * 256:(m + 1) * 256].T
        im["fcw"] = np.ascontiguousarray(
            cols.reshape(2, P, OUT_DIM).transpose(1, 0, 2).reshape(P, 2 * OUT_DIM)
        ).astype(ml_dtypes.bfloat16)
        in_maps.append(im)
    return in_maps


def kernel(**inputs) -> np.ndarray:
    from concourse.bass_utils import run_bass_kernel_spmd

    reps = int(os.environ.get("KERNEL_REPS", "1"))
    key = ("nc", reps)
    if key not in _cache:
        _cache[key] = _build(reps)
    nc = _cache[key]

    in_maps = _prep_inputs(inputs)
    res = run_bass_kernel_spmd(nc, in_maps, list(range(N_CORES)))
    if res.exec_time_ns is not None:
        print(f"HW exec time: {res.exec_time_ns} ns")
    acc = np.zeros((OUT_DIM, B), dtype=np.float32)
    for r in res.results:
        acc += r["out"]
    fc_b = np.asarray(inputs["fc_b"], dtype=np.float32)
    return (acc.T + fc_b[None, :]).astype(np.float32)


if __name__ == "__main__":
    sys.path.insert(0, "/root/problem")
    inputs = dict(np.load("/root/problem/inputs.npz"))
    expected = np.load("/root/problem/expected.npy")
    actual = kernel(**inputs)
    err = np.abs(actual - expected)
    scale = np.abs(expected).max()
    print(f"absmax err: {err.max():.6g}  scale: {scale:.6g}")
    print(f"Relative error: {err.max() / scale:.6g}")


# revision 5
# speedup vs baseline: 2.8231x; 1.7032x over previous
"""Trainium2 Bass kernel for the 3-layer LCN/GNN network, PE-dense strategy
with multi-way split AllGather/compute overlap and L0 row compaction.

As kernel_g, generalized: layer 0 computes its output columns in FOUR
quarter-passes (layer 1 in two half-passes), so the first quarter's
bias/ReLU/transpose/AllGather overlaps the remaining three quarters'
PE streaming, and each gathered piece's restage + next-layer k-tiles
start earlier. Finer column splits also deepen the L0 contraction-row
compaction: a quarter's S_0 columns hit only ~22% of the 16384 rows
(mean hits/row 0.25), so each (core, quarter) streams just KT0Q=32
k-tiles of compacted rows. Per-quarter transposed outputs are staged in
one tile and written to the DRAM shard with a single DMA.
"""

import os
import sys
import types

import numpy as np

try:  # pragma: no cover
    import antenv.axon_hooks  # noqa: F401
except Exception:
    _m = types.ModuleType("antenv.axon_hooks")
    _m.get_axon_ntff_profile_hook = lambda: None
    sys.modules["antenv.axon_hooks"] = _m

B, IN_DIM, K = 256, 16384, 16
DIMS = [8192, 4096, 2048]
PREV = [IN_DIM] + DIMS[:-1]
OUT_DIM = 3
N_CORES = 8
P = 128
HL = [4, 2, 1]     # column-split count per layer
KT0Q = 32          # compacted L0 k-tiles per quarter (mean 28.3, +8.9 sigma)
JJ = 4             # k-tiles per streamed S slab (256KB)

_cache = {}


def _perm(dim, H):
    """Row order of the gathered table for a layer with `dim` total nodes
    split H ways: piece-major [cores x piece0 | cores x piece1 | ...]."""
    sh = dim // N_CORES
    blk = sh // H
    return np.asarray(
        [c * sh + q * blk + j
         for q in range(H) for c in range(N_CORES) for j in range(blk)],
        dtype=np.int64)


def _build(reps: int = 1):
    import concourse.tile as tile
    from concourse import bacc, mybir

    nc = bacc.Bacc("TRN2", target_bir_lowering=False, debug=False,
                   num_devices=N_CORES)
    f32 = mybir.dt.float32
    bf16 = mybir.dt.bfloat16

    shard = [d // N_CORES for d in DIMS]  # 1024, 512, 256
    groups = [list(range(N_CORES))]
    kts = [KT0Q, PREV[1] // P, PREV[2] // P]  # k-tiles per pass

    t0 = nc.dram_tensor("t0", [HL[0] * KT0Q * P, B], bf16,
                        kind="ExternalInput")
    tn, shd = [], []
    for l in range(2):
        H = HL[l]
        tn.append([nc.dram_tensor(f"t{l + 1}p{h}", [DIMS[l] // H, B], bf16,
                                  addr_space="Shared") for h in range(H)])
        shd.append([nc.dram_tensor(f"sh{l + 1}p{h}", [shard[l] // H, B], bf16)
                    for h in range(H)])

    s_d = [nc.dram_tensor(f"s{l}", [HL[l] * kts[l] * P, shard[l] // HL[l]],
                          bf16, kind="ExternalInput") for l in range(3)]
    bias_d = [nc.dram_tensor(f"bias{l}", [1, shard[l]], bf16,
                             kind="ExternalInput") for l in range(3)]
    ones_d = nc.dram_tensor("ones", [1, P], bf16, kind="ExternalInput")
    fcw_d = nc.dram_tensor("fcw", [P, 2 * OUT_DIM], bf16, kind="ExternalInput")
    out_d = nc.dram_tensor("out", [OUT_DIM, B], f32, kind="ExternalOutput")

    with tile.TileContext(nc) as tc:
        with (
            tc.tile_pool(name="const", bufs=1) as cpool,
            tc.tile_pool(name="wstr", bufs=4) as wpool,
            tc.tile_pool(name="acts", bufs=2) as apool,
            tc.tile_pool(name="psum", bufs=1, space="PSUM") as ppool,
        ):
            a0 = cpool.tile([P, HL[0] * KT0Q, B], bf16, tag="a0")
            a1 = cpool.tile([P, DIMS[0] // P, B], bf16, tag="a1")
            a2 = cpool.tile([P, DIMS[1] // P, B], bf16, tag="a2")
            acts = [a0, a1, a2]
            nc.sync.dma_start(
                out=a0[:], in_=t0.ap().rearrange("(c p) b -> p c b", p=P))
            ones = cpool.tile([1, P], bf16, tag="ones")
            nc.sync.dma_start(out=ones[:], in_=ones_d.ap())
            bias_sb = []
            for l in range(3):
                bt = cpool.tile([1, shard[l]], bf16, tag=f"bias{l}")
                nc.sync.dma_start(out=bt[:], in_=bias_d[l].ap())
                bias_sb.append(bt)
            fcw_sb = cpool.tile([P, 2 * OUT_DIM], bf16, tag="fcw")
            nc.sync.dma_start(out=fcw_sb[:], in_=fcw_d.ap())
            act2T = cpool.tile([P, 2, B], bf16, tag="act2T")

            def emit_net():
                for l in range(3):
                    kt = kts[l]
                    H = HL[l]
                    nsh = shard[l] // H
                    at = acts[l]
                    for h in range(H):
                        pss = {}
                        for m in range(2):
                            pss[m] = ppool.tile([P, nsh], f32, tag=f"ps{m}",
                                                name=f"ps{m}")
                        for kj in range(kt // JJ):
                            st = wpool.tile([P, JJ, nsh], bf16, tag="S")
                            r0 = (h * kt + kj * JJ) * P
                            nc.sync.dma_start(
                                out=st[:],
                                in_=s_d[l][r0:r0 + JJ * P, :].rearrange(
                                    "(j p) n -> p j n", p=P))
                            for jj in range(JJ):
                                ki = kj * JJ + jj
                                kia = h * kt + ki if l == 0 else ki
                                for m in range(2):
                                    nc.tensor.matmul(
                                        out=pss[m][:],
                                        lhsT=at[:, kia, m * 128:(m + 1) * 128],
                                        rhs=st[:, jj, :],
                                        start=(ki == 0),
                                        stop=False,
                                    )
                        arows = []
                        for m in range(2):
                            nc.tensor.matmul(
                                out=pss[m][:],
                                lhsT=ones[:],
                                rhs=bias_sb[l][:, h * nsh:(h + 1) * nsh],
                                start=False,
                                stop=True,
                            )
                            arow = apool.tile([P, nsh], bf16, tag=f"ar{m}")
                            nc.scalar.activation(
                                out=arow[:],
                                in_=pss[m][:],
                                func=mybir.ActivationFunctionType.Relu,
                                scale=1.0,
                            )
                            arows.append(arow)
                        nt = nsh // P
                        if l < 2:
                            dst = apool.tile([P, nt, B], bf16, tag="T")
                        for n in range(nt):
                            if l < 2:
                                d0 = dst[:, n, 0:128]
                                d1 = dst[:, n, 128:256]
                            else:
                                g = h * nt + n
                                d0 = act2T[:, g, 0:128]
                                d1 = act2T[:, g, 128:256]
                            nc.sync.dma_start(
                                out=d0, in_=arows[0][:, n * P:(n + 1) * P],
                                transpose=True)
                            nc.sync.dma_start(
                                out=d1, in_=arows[1][:, n * P:(n + 1) * P],
                                transpose=True)
                        if l < 2:
                            nc.sync.dma_start(
                                out=shd[l][h].ap().rearrange(
                                    "(n p) b -> p n b", p=P),
                                in_=dst[:])
                            nc.gpsimd.collective_compute(
                                "AllGather",
                                mybir.AluOpType.bypass,
                                groups,
                                ins=[shd[l][h].ap()],
                                outs=[tn[l][h].ap()],
                            )
                            ktn_h = DIMS[l] // H // P
                            nc.sync.dma_start(
                                out=acts[l + 1][:, h * ktn_h:(h + 1) * ktn_h, :],
                                in_=tn[l][h].ap().rearrange(
                                    "(c p) b -> p c b", p=P))

                ps = ppool.tile([OUT_DIM, B], f32, tag="fc")
                for t in range(2):
                    nc.tensor.matmul(
                        out=ps[:],
                        lhsT=fcw_sb[:, t * OUT_DIM:(t + 1) * OUT_DIM],
                        rhs=act2T[:, t, :],
                        start=(t == 0),
                        stop=(t == 1),
                    )
                fin = apool.tile([OUT_DIM, B], f32, tag="fin")
                nc.vector.tensor_copy(out=fin[:], in_=ps[:])
                nc.sync.dma_start(out=out_d.ap(), in_=fin[:])

            for r in range(reps):
                if r:
                    tc.strict_bb_all_engine_barrier()
                emit_net()

    nc.compile()
    return nc


def _prep_inputs(inputs):
    import ml_dtypes

    shard = [d // N_CORES for d in DIMS]
    x = np.asarray(inputs["x"], dtype=np.float32)
    t0 = np.ascontiguousarray(x.T).astype(ml_dtypes.bfloat16)
    fcw = np.asarray(inputs["fc_w"], dtype=np.float32)
    ones = np.ones((1, P), dtype=ml_dtypes.bfloat16)
    perms = [None, _perm(DIMS[0], HL[0]), _perm(DIMS[1], HL[1])]

    in_maps = []
    for m in range(N_CORES):
        im = {"ones": ones}
        for l, d in enumerate(DIMS):
            knn = np.asarray(inputs[f"knn{l}"], dtype=np.int64)
            w = np.asarray(inputs[f"w{l}"], dtype=np.float32)
            b = np.asarray(inputs[f"b{l}"], dtype=np.float32).reshape(d)
            lo = m * shard[l]
            nodes = np.arange(lo, lo + shard[l])
            s32 = np.zeros((PREV[l], shard[l]), dtype=np.float32)
            rows = knn[nodes].ravel()
            cols = np.repeat(np.arange(shard[l]), K)
            np.add.at(s32, (rows, cols), w[nodes].ravel())
            if perms[l] is not None:
                s32 = s32[perms[l]]  # match the gathered-table row order
            H = HL[l]
            nsh = shard[l] // H
            if l == 0:
                sblk, tblk = [], []
                for h in range(H):
                    sub = s32[:, h * nsh:(h + 1) * nsh]
                    used = np.flatnonzero(sub.any(axis=1))
                    assert len(used) <= KT0Q * P, len(used)
                    sc = np.zeros((KT0Q * P, nsh), dtype=np.float32)
                    sc[:len(used)] = sub[used]
                    tc_ = np.zeros((KT0Q * P, B), dtype=ml_dtypes.bfloat16)
                    tc_[:len(used)] = t0[used]
                    sblk.append(sc)
                    tblk.append(tc_)
                im["t0"] = np.ascontiguousarray(np.concatenate(tblk, axis=0))
                s32 = np.concatenate(sblk, axis=0)
            elif H > 1:
                s32 = np.concatenate(
                    [s32[:, h * nsh:(h + 1) * nsh] for h in range(H)], axis=0)
            im[f"s{l}"] = np.ascontiguousarray(s32).astype(ml_dtypes.bfloat16)
            im[f"bias{l}"] = b[lo:lo + shard[l]].reshape(1, -1).astype(
                ml_dtypes.bfloat16)
        cols = fcw[:, m * 256:(m + 1) * 256].T
        im["fcw"] = np.ascontiguousarray(
            cols.reshape(2, P, OUT_DIM).transpose(1, 0, 2).reshape(P, 2 * OUT_DIM)
        ).astype(ml_dtypes.bfloat16)
        in_maps.append(im)
    return in_maps


def kernel(**inputs) -> np.ndarray:
    from concourse.bass_utils import run_bass_kernel_spmd

    reps = int(os.environ.get("KERNEL_REPS", "1"))
    key = ("nc", reps)
    if key not in _cache:
        _cache[key] = _build(reps)
    nc = _cache[key]

    in_maps = _prep_inputs(inputs)
    res = run_bass_kernel_spmd(nc, in_maps, list(range(N_CORES)))
    if res.exec_time_ns is not None:
        print(f"HW exec time: {res.exec_time_ns} ns")
    acc = np.zeros((OUT_DIM, B), dtype=np.float32)
    for r in res.results:
        acc += r["out"]
    fc_b = np.asarray(inputs["fc_b"], dtype=np.float32)
    return (acc.T + fc_b[None, :]).astype(np.float32)


if __name__ == "__main__":
    sys.path.insert(0, "/root/problem")
    inputs = dict(np.load("/root/problem/inputs.npz"))
    expected = np.load("/root/problem/expected.npy")
    actual = kernel(**inputs)
    err = np.abs(actual - expected)
    scale = np.abs(expected).max()
    print(f"absmax err: {err.max():.6g}  scale: {scale:.6g}")
    print(f"Relative error: {err.max() / scale:.6g}")
